# revision 2
# baseline (speedup 1.0000x reference)
"""NetVLAD pooling kernel v3 for Trainium2 (Bass/Tile), SPMD over 8 NeuronCores.

Structure (per sample, S=16384 positions in 16 groups of 1024):

  PREPASS (per group g):     dma xf -> gpsimd cast xh_all slice (persistent,
    32KB/partition) -> x2 (ACT square) -> 8 ssq matmuls (stationary=x2 tile,
    moving=ones column) into ssqp_all[128, 128] PSUM (one col per tile).
  BATCHED NORM (once per sample):  normv_all = sqrt(ssqp_all) (1 ACT op),
    rnorm_all = 1/normv_all (1 DVE op); ONE PE transpose of normv_all
    -> PSUM -> DVE evac to f16 nrt [128,128] (row r = norms of tile r).
  MAIN (per group g):
    bias-mm (PE):   lgp[s,(t,k)]  = norm[s,t]*b[k]   (start=True)
                    lhsT = nrt [128,128] (all tiles' norm rows, contract=128),
                    rhs  = bias_all[:, g*tpg*K:...] -- per-group block-select
                    bias constant (b[k] at row g*tpg+t, zero elsewhere)
    mm1a x8 (PE):   xT_psum = xh_t^T          (identity moving)
    mm1b x8 (PE):   lgp += fcw @ xh_t         (start=False: accumulates onto
                                               the bias -> lgp = l + norm*b)
    t1 (1 DVE op):  t1 = lgp * rnorm_b  (f16)  == l*rnorm + b   [fuses
                    normalize-scale AND bias application]
    exp (1 ACT op): E = exp(t1)
    reduce (1 op):  sume[s,t] = sum_k E   (DVE or GPSIMD, knob)
    rsum=1/sume; qsc=rsum*rnorm; wt = E*qsc_b (DVE)
    xt copy (DVE/ACT split) + norm column; mm2 x8 accumulate vlad.
  EPILOGUE: as before; global L2 norm folded to *0.125.

PSUM: xT [128,1024] x2bufs (4 banks), lg [128,512] x2 (2), misc 1 bank
(ssqp_all[*,0:128] + normT_A[0:64,128:256] + normT_B[0:64,256:384]),
vlad 1 bank. Total 8.
"""

import contextlib
import numpy as np

import concourse.bacc as bacc
import concourse.bass as bass
import concourse.mybir as mybir
import concourse.tile as tile

N, C, S, K = 16, 128, 16384, 64
N_CORES = 8
N_PER_CORE = N // N_CORES  # 2

F32 = mybir.dt.float32
F16 = mybir.dt.float16
AF = mybir.ActivationFunctionType
ALU = mybir.AluOpType
AX = mybir.AxisListType

TILE = 128

DEFAULT_OPTS = dict(
    group=1024,           # positions per group (tpg = group//128)
    xt_dve_cols=32,       # cols of the x^T copy done on DVE (rest ACT)
    x2_engine="act",      # dve | act | gpsimd
    reduce_engine="dve",  # dve | gpsimd
    xT_bufs=2,
    lg_bufs=2,
    sbuf_bufs=0,
    ablate=(),            # subset of {mm1,ssq,bias,t1,exp,reduce,wt,xtcopy,
                          #            x2,cast,dma,mm2}
)


def build_nc(n_samples=N_PER_CORE, s_len=S, finalize=True, repeat=1, opts=None):
    o = dict(DEFAULT_OPTS)
    if opts:
        o.update(opts)
    group = o["group"]
    tpg = group // TILE
    n_groups = s_len // group
    assert n_groups <= 16, "norm_rowT batching assumes <=16 groups/sample"

    nc = bacc.Bacc("TRN2", target_bir_lowering=False, debug=False)

    x_d = nc.dram_tensor("x", [n_samples, C, s_len], F32, kind="ExternalInput")
    rhsc_d = nc.dram_tensor("rhs_const", [128, 192], F16, kind="ExternalInput")
    bblk_d = nc.dram_tensor(
        "bias_blk", [128, n_groups * tpg * K], F16, kind="ExternalInput")
    cent_d = nc.dram_tensor("centroids", [K, C], F32, kind="ExternalInput")
    ident_d = nc.dram_tensor("ident", [128, 128], F32, kind="ExternalInput")
    out_d = nc.dram_tensor("out", [n_samples, K, C], F32, kind="ExternalOutput")

    with tile.TileContext(nc) as tc:
        with (
            tc.tile_pool(name="const", bufs=1) as const_pool,
            tc.tile_pool(name="xf", bufs=3 + o["sbuf_bufs"]) as x_pool,
            tc.tile_pool(name="xall", bufs=2) as xall_pool,
            tc.tile_pool(name="x2", bufs=2 + o["sbuf_bufs"]) as x2_pool,
            tc.tile_pool(name="xt", bufs=2 + o["sbuf_bufs"]) as xt_pool,
            tc.tile_pool(name="ew", bufs=2 + o["sbuf_bufs"]) as ew_pool,
            tc.tile_pool(name="sm", bufs=2 + o["sbuf_bufs"]) as sm_pool,
            tc.tile_pool(name="ep", bufs=1) as ep_pool,
        ):
            rhs_const = const_pool.tile([128, 192], F16, tag="rhsc")
            nc.sync.dma_start(out=rhs_const[:], in_=rhsc_d.ap())
            bias_blk = const_pool.tile(
                [128, n_groups * tpg * K], F16, tag="bblk")
            nc.sync.dma_start(out=bias_blk[:], in_=bblk_d.ap())
            cent_sb = const_pool.tile([K, C], F32, tag="cent")
            nc.sync.dma_start(out=cent_sb[:], in_=cent_d.ap())
            ident_f32 = const_pool.tile([128, 128], F32, tag="ident")
            nc.sync.dma_start(out=ident_f32[:], in_=ident_d.ap())
            ones_col_f16 = const_pool.tile([128, 1], F16, tag="ones_col")
            nc.vector.memset(ones_col_f16[:], 1.0)
            # nrt: transposed norm rows, one per sample parity
            nrtA = const_pool.tile([128, 128], F16, tag="nrtA")
            nc.vector.memset(nrtA[:], 0.0)
            nrtB = const_pool.tile([128, 128], F16, tag="nrtB")
            nc.vector.memset(nrtB[:], 0.0)

            with (
                tc.tile_pool(name="xTp", bufs=o["xT_bufs"], space="PSUM") as xT_pool,
                tc.tile_pool(name="lgp", bufs=o["lg_bufs"], space="PSUM") as lg_pool,
                tc.tile_pool(name="misc", bufs=1, space="PSUM") as misc_pool,
                tc.tile_pool(name="vladp", bufs=1, space="PSUM") as vlad_pool,
            ):
                env = dict(
                    o=o, group=group, tpg=tpg, n_groups=n_groups,
                    n_samples=n_samples,
                    x_pool=x_pool, xall_pool=xall_pool, x2_pool=x2_pool,
                    xT_pool=xT_pool, lg_pool=lg_pool, misc_pool=misc_pool,
                    vlad_pool=vlad_pool, xt_pool=xt_pool, ew_pool=ew_pool,
                    sm_pool=sm_pool, ep_pool=ep_pool,
                    rhs_const=rhs_const, bias_blk=bias_blk,
                    nrtA=nrtA, nrtB=nrtB,
                    ones_col_f16=ones_col_f16, cent_sb=cent_sb,
                    ident_f32=ident_f32,
                )
                loop_ctx = (tc.For_i(0, repeat, 1) if repeat > 1
                            else contextlib.nullcontext())
                with loop_ctx:
                    _main_body(nc, x_d.ap(), out_d.ap(), env)

    if finalize:
        nc.finalize()
    return nc


def _main_body(nc, x_ap, out_ap, env):
    o = env["o"]
    group, tpg = env["group"], env["tpg"]
    n_samples, n_groups = env["n_samples"], env["n_groups"]
    x_pool = env["x_pool"]; xall_pool = env["xall_pool"]; x2_pool = env["x2_pool"]
    xT_pool = env["xT_pool"]; lg_pool = env["lg_pool"]
    misc_pool = env["misc_pool"]; vlad_pool = env["vlad_pool"]
    xt_pool = env["xt_pool"]; ew_pool = env["ew_pool"]; sm_pool = env["sm_pool"]
    ep_pool = env["ep_pool"]
    rhs_const = env["rhs_const"]; bias_blk = env["bias_blk"]
    nrtA = env["nrtA"]; nrtB = env["nrtB"]
    ones_col_f16 = env["ones_col_f16"]; cent_sb = env["cent_sb"]
    ident_f32 = env["ident_f32"]
    xt_dve = o["xt_dve_cols"]
    ab = set(o.get("ablate") or ())
    s_len = group * n_groups

    for n in range(n_samples):
        vb = vlad_pool.tile([128, 132], F32)
        vlad_psum = vb[0:K, 0:129]
        asum_col = vb[0:K, 128:129]

        # misc bank: ssqp_all + 2 transpose scratches
        mb_t = misc_pool.tile([128, 384], F32)
        ssqp_all = mb_t[:, 0:128]

        xh_all = xall_pool.tile([128, s_len], F16)

        # ---------------- prepass ----------------
        for g in range(n_groups):
            first_g = g == 0
            keep = lambda what: what not in ab or first_g  # noqa: E731

            xf = x_pool.tile([128, group], F32)
            if "dma" in ab and not first_g:
                nc.sync.dma_start(out=xf[:, 0:4], in_=x_ap[n][:, 0:4])
            else:
                nc.sync.dma_start(
                    out=xf[:], in_=x_ap[n][:, g * group:(g + 1) * group])

            xh = xh_all[:, g * group:(g + 1) * group]
            if keep("cast"):
                nc.gpsimd.tensor_copy(xh, xf[:])
            else:
                nc.gpsimd.tensor_copy(xh_all[:, g * group:g * group + 8],
                                      xf[:, 0:8])
            x2 = x2_pool.tile([128, group], F16)
            if not keep("x2"):
                nc.vector.tensor_mul(x2[:, 0:8], xh[:, 0:8], xh[:, 0:8])
            else:
                if o["x2_engine"] == "dve":
                    nc.vector.tensor_mul(x2[:], xh, xh)
                elif o["x2_engine"] == "act":
                    nc.scalar.square(x2[:], xh)
                else:
                    nc.gpsimd.tensor_mul(x2[:], xh, xh)
            for t in (range(tpg) if keep("ssq") else range(1)):
                nc.tensor.matmul(
                    ssqp_all[:, g * tpg + t: g * tpg + t + 1],
                    lhsT=x2[:, t * TILE:(t + 1) * TILE],
                    rhs=ones_col_f16[:],
                    start=True, stop=True, skip_group_check=True,
                )

        # ---------------- batched norm ----------------
        normv_all = sm_pool.tile([128, 128], F32, tag="normv")
        nc.scalar.activation(normv_all[:, 0:n_groups * tpg],
                             ssqp_all[:, 0:n_groups * tpg], func=AF.Sqrt)
        rnorm_all = sm_pool.tile([128, 128], F32, tag="rnorm")
        nc.vector.reciprocal(rnorm_all[:, 0:n_groups * tpg],
                             normv_all[:, 0:n_groups * tpg])

        ntiles = n_groups * tpg  # <=128
        nrt = nrtA if n % 2 == 0 else nrtB
        if "bias" not in ab:
            ntp = mb_t[0:ntiles, 128:256]
            nc.tensor.matmul(
                ntp, lhsT=normv_all[:, 0:ntiles], rhs=ident_f32[:],
                start=True, stop=True, is_transpose=True,
                skip_group_check=True,
            )
            nc.vector.tensor_copy(nrt[0:ntiles, :], ntp)

        # ---------------- main loop ----------------
        for g in range(n_groups):
            first_g = g == 0
            keep = lambda what: what not in ab or first_g  # noqa: E731

            xTp = xT_pool.tile([128, tpg * 128], F32)
            lgp = lg_pool.tile([128, tpg * K], F32)
            lg_3d = lgp[:].rearrange("p (t x) -> p t x", t=tpg)
            xT_3d = xTp[:].rearrange("p (t x) -> p t x", t=tpg)
            rnorm_g = rnorm_all[:, g * tpg:(g + 1) * tpg]
            normv_g = normv_all[:, g * tpg:(g + 1) * tpg]

            if keep("bias"):
                nc.tensor.matmul(
                    lgp[:],
                    lhsT=nrt[:],
                    rhs=bias_blk[:, g * tpg * K:(g + 1) * tpg * K],
                    start=True, stop=False, skip_group_check=True,
                )
            for t in (range(tpg) if keep("mm1") else range(1)):
                nc.tensor.matmul(
                    xTp[:, t * 128:(t + 1) * 128],
                    lhsT=xh_all[:, g * group + t * TILE:
                                g * group + (t + 1) * TILE],
                    rhs=rhs_const[:, 0:128],
                    start=True, stop=True, skip_group_check=True,
                )
            mm1b_last = (tpg - 1) if keep("mm1") else 0
            for t in (range(tpg) if keep("mm1") else range(1)):
                nc.tensor.matmul(
                    lgp[:, t * K:(t + 1) * K],
                    lhsT=xh_all[:, g * group + t * TILE:
                                g * group + (t + 1) * TILE],
                    rhs=rhs_const[:, 128:192],
                    start=(not keep("bias") and t == 0), stop=(t == mm1b_last),
                    skip_group_check=True,
                )

            # t1 = lgp * rnorm_b  (fuses softmax scale AND bias application)
            t1 = ew_pool.tile([128, tpg * K], F16, tag="t1")
            t1_3d = t1[:].rearrange("p (t x) -> p t x", t=tpg)
            rnorm_b = rnorm_g.unsqueeze(-1).broadcast_to((128, tpg, K))
            if keep("t1"):
                nc.vector.tensor_mul(t1_3d, lg_3d, rnorm_b)
            else:
                nc.vector.tensor_mul(t1[:, 0:8], lgp[:, 0:8], lgp[:, 0:8])

            E = ew_pool.tile([128, tpg * K], F16, tag="E")
            if keep("exp"):
                nc.scalar.activation(E[:], t1[:], func=AF.Exp)
            else:
                nc.scalar.activation(E[:, 0:8], t1[:, 0:8], func=AF.Exp)
            E_3d = E[:].rearrange("p (t x) -> p t x", t=tpg)

            sume = sm_pool.tile([128, tpg], F32, tag="sume")
            if keep("reduce"):
                eng = nc.vector if o["reduce_engine"] == "dve" else nc.gpsimd
                eng.tensor_reduce(sume[:], E_3d, axis=AX.X, op=ALU.add)
            else:
                nc.vector.tensor_reduce(
                    sume[:, 0:1], E_3d[:, 0:1, :], axis=AX.X, op=ALU.add)
            rsum = sm_pool.tile([128, tpg], F32, tag="rsum")
            nc.vector.reciprocal(rsum[:], sume[:])
            qsc = sm_pool.tile([128, tpg], F32, tag="qsc")
            nc.vector.tensor_mul(qsc[:], rsum[:], rnorm_g)

            wt = ew_pool.tile([128, tpg * K], F16, tag="wt")
            if keep("wt"):
                wt_3d = wt[:].rearrange("p (t x) -> p t x", t=tpg)
                q_b = qsc[:].unsqueeze(-1).broadcast_to((128, tpg, K))
                nc.vector.tensor_mul(wt_3d, E_3d, q_b)
            else:
                nc.vector.tensor_mul(wt[:, 0:8], E[:, 0:8], E[:, 0:8])

            xt = xt_pool.tile([128, tpg * 132], F16)
            xt_3d = xt[:].rearrange("p (t x) -> p t x", t=tpg)
            if keep("xtcopy"):
                if xt_dve > 0:
                    nc.vector.tensor_copy(
                        xt_3d[:, :, 0:xt_dve], xT_3d[:, :, 0:xt_dve])
                if xt_dve < 128:
                    nc.scalar.copy(
                        xt_3d[:, :, xt_dve:128], xT_3d[:, :, xt_dve:128])
                nc.vector.tensor_copy(
                    xt_3d[:, :, 128:129], normv_g.unsqueeze(-1))
            else:
                nc.vector.tensor_copy(xt_3d[:, 0:1, 0:129], xT_3d[:, 0:1, 0:129])

            mm2_tiles = range(tpg)
            if "mm2" in ab:
                mm2_tiles = range(1) if (g == 0 or g == n_groups - 1) else range(0)
            for t in mm2_tiles:
                first = (g == 0 and t == 0)
                last = ("mm2" in ab and g == n_groups - 1 and t == 0) or \
                       (g == n_groups - 1 and t == tpg - 1)
                nc.tensor.matmul(
                    vlad_psum,
                    lhsT=wt[:, t * K:(t + 1) * K],
                    rhs=xt[:, t * 132: t * 132 + 129],
                    start=first, stop=last, skip_group_check=True,
                )

        # -------- epilogue for sample n --------
        acs = ep_pool.tile([K, C], F32, tag="acs")
        nc.vector.tensor_scalar_mul(acs[:], cent_sb[:], asum_col)
        v = ep_pool.tile([K, C], F32, tag="v")
        nc.vector.tensor_sub(v[:], vb[0:K, 0:128], acs[:])
        v2 = ep_pool.tile([K, C], F32, tag="v2")
        nc.vector.tensor_mul(v2[:], v[:], v[:])
        ssqv = sm_pool.tile([K, 1], F32, tag="ssqv")
        nc.vector.tensor_reduce(ssqv[:], v2[:], axis=AX.X, op=ALU.add)
        nv = sm_pool.tile([K, 1], F32, tag="nv")
        nc.scalar.activation(nv[:], ssqv[:], func=AF.Sqrt)
        rnv = sm_pool.tile([K, 1], F32, tag="rnv")
        nc.vector.reciprocal(rnv[:], nv[:])
        rnv8 = sm_pool.tile([K, 1], F32, tag="rnv8")
        nc.vector.tensor_scalar_mul(rnv8[:], rnv[:], 0.125)
        o_t = ep_pool.tile([K, C], F32, tag="o")
        nc.vector.tensor_scalar_mul(o_t[:], v[:], rnv8[:])
        nc.sync.dma_start(out=out_ap[n], in_=o_t[:])


def host_consts(fc_w, fc_b, opts=None):
    o = dict(DEFAULT_OPTS)
    if opts:
        o.update(opts)
    tpg = o["group"] // TILE
    rhs_const = np.zeros((128, 192), dtype=np.float16)
    rhs_const[:, 0:128] = np.eye(128, dtype=np.float16)
    rhs_const[:, 128:192] = fc_w.astype(np.float16).T  # [C, K]
    n_groups = S // o["group"]
    bias_blk = np.zeros((128, n_groups * tpg * K), dtype=np.float16)
    b16 = fc_b.reshape(-1).astype(np.float16)
    for g in range(n_groups):
        for t in range(tpg):
            bias_blk[g * tpg + t,
                     (g * tpg + t) * K:(g * tpg + t + 1) * K] = b16
    ident = np.eye(128, dtype=np.float32)
    return dict(rhs_const=rhs_const, bias_blk=bias_blk, ident=ident)


def make_in_maps(opts=None, seed=0):
    rng = np.random.default_rng(seed)
    fc_w = rng.standard_normal((K, C)).astype(np.float32)
    fc_b = rng.standard_normal((K,)).astype(np.float32)
    consts = host_consts(fc_w, fc_b, opts)
    ins = {
        "x": rng.standard_normal((N_PER_CORE, C, S)).astype(np.float32),
        "centroids": rng.random((K, C)).astype(np.float32),
        **consts,
    }
    return [ins] * N_CORES


def kernel(x, fc_w, fc_b, centroids):
    """Full-input entry point: shards over 8 cores, returns [N, K*C] float32."""
    from concourse.bass_utils import run_bass_kernel_spmd

    x = np.ascontiguousarray(np.asarray(x, dtype=np.float32))
    fc_w = np.ascontiguousarray(np.asarray(fc_w, dtype=np.float32))
    fc_b = np.ascontiguousarray(np.asarray(fc_b, dtype=np.float32))
    centroids = np.ascontiguousarray(np.asarray(centroids, dtype=np.float32))

    consts = host_consts(fc_w, fc_b)
    nc = build_nc(N_PER_CORE, S)
    core_ids = list(range(N_CORES))
    in_maps = []
    for i in core_ids:
        in_maps.append({
            "x": x[i * N_PER_CORE:(i + 1) * N_PER_CORE],
            "centroids": centroids,
            **consts,
        })
    last_exc = None
    for attempt in range(4):
        try:
            res = run_bass_kernel_spmd(nc, in_maps, core_ids)
            break
        except Exception as e:  # noqa: BLE001
            last_exc = e
            if attempt == 3:
                raise
            import time as _time
            _time.sleep(45)
    outs = [res.results[i]["out"].reshape(N_PER_CORE, K * C) for i in range(N_CORES)]
    return np.concatenate(outs, axis=0)


# revision 3
# speedup vs baseline: 1.0882x; 1.0882x over previous
"""NetVLAD pooling kernel v3 for Trainium2 (Bass/Tile), SPMD over 8 NeuronCores.

Structure (per sample, S=16384 positions in 16 groups of 1024):

  PREPASS (per group g):     dma xf -> gpsimd cast xh_all slice (persistent,
    32KB/partition) -> x2 (ACT square) -> 8 ssq matmuls (stationary=x2 tile,
    moving=ones column) into ssqp_all[128, 128] PSUM (one col per tile).
  BATCHED NORM (once per sample):  normv_all = sqrt(ssqp_all) (1 ACT op),
    rnorm_all = 1/normv_all (1 DVE op); ONE PE transpose of normv_all
    -> PSUM -> DVE evac to f16 nrt [128,128] (row r = norms of tile r).
  MAIN (per group g):
    bias-mm (PE):   lgp[s,(t,k)]  = norm[s,t]*b[k]   (start=True)
                    lhsT = nrt [128,128] (all tiles' norm rows, contract=128),
                    rhs  = bias_all[:, g*tpg*K:...] -- per-group block-select
                    bias constant (b[k] at row g*tpg+t, zero elsewhere)
    mm1a x8 (PE):   xT_psum = xh_t^T          (identity moving)
    mm1b x8 (PE):   lgp += fcw @ xh_t         (start=False: accumulates onto
                                               the bias -> lgp = l + norm*b)
    t1 (1 DVE op):  t1 = lgp * rnorm_b  (f16)  == l*rnorm + b   [fuses
                    normalize-scale AND bias application]
    exp (1 ACT op): E = exp(t1)
    reduce (1 op):  sume[s,t] = sum_k E   (DVE or GPSIMD, knob)
    rsum=1/sume; qsc=rsum*rnorm; wt = E*qsc_b (DVE)
    xt copy (DVE/ACT split) + norm column; mm2 x8 accumulate vlad.
  EPILOGUE: as before; global L2 norm folded to *0.125.

PSUM: xT [128,1024] x2bufs (4 banks), lg [128,512] x2 (2), misc 1 bank
(ssqp_all[*,0:128] + normT_A[0:64,128:256] + normT_B[0:64,256:384]),
vlad 1 bank. Total 8.
"""

import contextlib
import numpy as np

import concourse.bacc as bacc
import concourse.bass as bass
import concourse.mybir as mybir
import concourse.tile as tile

N, C, S, K = 16, 128, 16384, 64
N_CORES = 8
N_PER_CORE = N // N_CORES  # 2

F32 = mybir.dt.float32
F16 = mybir.dt.float16
AF = mybir.ActivationFunctionType
ALU = mybir.AluOpType
AX = mybir.AxisListType

TILE = 128

DEFAULT_OPTS = dict(
    group=1024,           # positions per group (tpg = group//128)
    dma_groups=1,         # groups loaded per dma_start (1 = one per group)
    xt_dve_cols=0,        # cols of the x^T copy done on DVE (rest ACT)
    x2_engine="act",      # dve | act | gpsimd
    reduce_engine="dve",  # dve | gpsimd
    cast_dve_cols=0,      # cols of each group's f32->f16 cast done on DVE
    xT_bufs=2,
    lg_bufs=2,
    sbuf_bufs=1,
    ablate=(),            # subset of {mm1,ssq,bias,t1,exp,reduce,wt,xtcopy,
                          #            x2,cast,dma,mm2}
)


def build_nc(n_samples=N_PER_CORE, s_len=S, finalize=True, repeat=1, opts=None):
    o = dict(DEFAULT_OPTS)
    if opts:
        o.update(opts)
    group = o["group"]
    tpg = group // TILE
    n_groups = s_len // group
    assert n_groups * tpg <= 128, "norm_rowT batching needs <=128 tiles/sample"

    nc = bacc.Bacc("TRN2", target_bir_lowering=False, debug=False)

    x_d = nc.dram_tensor("x", [n_samples, C, s_len], F32, kind="ExternalInput")
    rhsc_d = nc.dram_tensor("rhs_const", [128, 192], F16, kind="ExternalInput")
    bblk_d = nc.dram_tensor(
        "bias_blk", [128, n_groups * tpg * K], F16, kind="ExternalInput")
    cent_d = nc.dram_tensor("centroids", [K, C], F32, kind="ExternalInput")
    ident_d = nc.dram_tensor("ident", [128, 128], F32, kind="ExternalInput")
    out_d = nc.dram_tensor("out", [n_samples, K, C], F32, kind="ExternalOutput")

    with tile.TileContext(nc) as tc:
        with (
            tc.tile_pool(name="const", bufs=1) as const_pool,
            tc.tile_pool(name="xf", bufs=3 + o["sbuf_bufs"]) as x_pool,
            tc.tile_pool(name="xall", bufs=2) as xall_pool,
            tc.tile_pool(name="x2", bufs=2 + o["sbuf_bufs"]) as x2_pool,
            tc.tile_pool(name="xt", bufs=2 + o["sbuf_bufs"]) as xt_pool,
            tc.tile_pool(name="ew", bufs=2 + o["sbuf_bufs"]) as ew_pool,
            tc.tile_pool(name="sm", bufs=2 + o["sbuf_bufs"]) as sm_pool,
            tc.tile_pool(name="ep", bufs=1) as ep_pool,
        ):
            rhs_const = const_pool.tile([128, 192], F16, tag="rhsc")
            nc.sync.dma_start(out=rhs_const[:], in_=rhsc_d.ap())
            bias_blk = const_pool.tile(
                [128, n_groups * tpg * K], F16, tag="bblk")
            nc.sync.dma_start(out=bias_blk[:], in_=bblk_d.ap())
            cent_sb = const_pool.tile([K, C], F32, tag="cent")
            nc.sync.dma_start(out=cent_sb[:], in_=cent_d.ap())
            ident_f32 = const_pool.tile([128, 128], F32, tag="ident")
            nc.sync.dma_start(out=ident_f32[:], in_=ident_d.ap())
            ones_col_f16 = const_pool.tile([128, 1], F16, tag="ones_col")
            nc.vector.memset(ones_col_f16[:], 1.0)
            # nrt: transposed norm rows, one per sample parity
            nrtA = const_pool.tile([128, 128], F16, tag="nrtA")
            nc.vector.memset(nrtA[:], 0.0)
            nrtB = const_pool.tile([128, 128], F16, tag="nrtB")
            nc.vector.memset(nrtB[:], 0.0)

            with (
                tc.tile_pool(name="xTp", bufs=o["xT_bufs"], space="PSUM") as xT_pool,
                tc.tile_pool(name="lgp", bufs=o["lg_bufs"], space="PSUM") as lg_pool,
                tc.tile_pool(name="misc", bufs=1, space="PSUM") as misc_pool,
                tc.tile_pool(name="vladp", bufs=1, space="PSUM") as vlad_pool,
            ):
                env = dict(
                    o=o, group=group, tpg=tpg, n_groups=n_groups,
                    n_samples=n_samples,
                    x_pool=x_pool, xall_pool=xall_pool, x2_pool=x2_pool,
                    xT_pool=xT_pool, lg_pool=lg_pool, misc_pool=misc_pool,
                    vlad_pool=vlad_pool, xt_pool=xt_pool, ew_pool=ew_pool,
                    sm_pool=sm_pool, ep_pool=ep_pool,
                    rhs_const=rhs_const, bias_blk=bias_blk,
                    nrtA=nrtA, nrtB=nrtB,
                    ones_col_f16=ones_col_f16, cent_sb=cent_sb,
                    ident_f32=ident_f32,
                )
                loop_ctx = (tc.For_i(0, repeat, 1) if repeat > 1
                            else contextlib.nullcontext())
                with loop_ctx:
                    _main_body(nc, x_d.ap(), out_d.ap(), env)

    if finalize:
        nc.finalize()
    return nc


def _main_body(nc, x_ap, out_ap, env):
    o = env["o"]
    group, tpg = env["group"], env["tpg"]
    n_samples, n_groups = env["n_samples"], env["n_groups"]
    x_pool = env["x_pool"]; xall_pool = env["xall_pool"]; x2_pool = env["x2_pool"]
    xT_pool = env["xT_pool"]; lg_pool = env["lg_pool"]
    misc_pool = env["misc_pool"]; vlad_pool = env["vlad_pool"]
    xt_pool = env["xt_pool"]; ew_pool = env["ew_pool"]; sm_pool = env["sm_pool"]
    ep_pool = env["ep_pool"]
    rhs_const = env["rhs_const"]; bias_blk = env["bias_blk"]
    nrtA = env["nrtA"]; nrtB = env["nrtB"]
    ones_col_f16 = env["ones_col_f16"]; cent_sb = env["cent_sb"]
    ident_f32 = env["ident_f32"]
    xt_dve = o["xt_dve_cols"]
    ab = set(o.get("ablate") or ())
    s_len = group * n_groups

    for n in range(n_samples):
        vb = vlad_pool.tile([128, 132], F32)
        vlad_psum = vb[0:K, 0:129]
        asum_col = vb[0:K, 128:129]

        # misc bank: ssqp_all + 2 transpose scratches
        mb_t = misc_pool.tile([128, 384], F32)
        ssqp_all = mb_t[:, 0:128]

        xh_all = xall_pool.tile([128, s_len], F16)

        # ---------------- prepass ----------------
        dgr = o["dma_groups"]
        xf_cur = None
        for g in range(n_groups):
            first_g = g == 0
            keep = lambda what: what not in ab or first_g  # noqa: E731

            if g % dgr == 0:
                xf_cur = x_pool.tile([128, group * dgr], F32)
                if "dma" in ab and not first_g:
                    nc.sync.dma_start(out=xf_cur[:, 0:4], in_=x_ap[n][:, 0:4])
                else:
                    nc.sync.dma_start(
                        out=xf_cur[:],
                        in_=x_ap[n][:, g * group:(g + dgr) * group])
            xf = xf_cur[:, (g % dgr) * group:(g % dgr + 1) * group]

            xh = xh_all[:, g * group:(g + 1) * group]
            if keep("cast"):
                cd = o["cast_dve_cols"]
                if cd > 0:
                    nc.vector.tensor_copy(
                        xh_all[:, g * group:g * group + cd], xf[:, 0:cd])
                if cd < group:
                    nc.gpsimd.tensor_copy(
                        xh_all[:, g * group + cd:(g + 1) * group],
                        xf[:, cd:group])
            else:
                nc.gpsimd.tensor_copy(xh_all[:, g * group:g * group + 8],
                                      xf[:, 0:8])
            x2 = x2_pool.tile([128, group], F16)
            if not keep("x2"):
                nc.vector.tensor_mul(x2[:, 0:8], xh[:, 0:8], xh[:, 0:8])
            else:
                if o["x2_engine"] == "dve":
                    nc.vector.tensor_mul(x2[:], xh, xh)
                elif o["x2_engine"] == "act":
                    nc.scalar.square(x2[:], xh)
                else:
                    nc.gpsimd.tensor_mul(x2[:], xh, xh)
            for t in (range(tpg) if keep("ssq") else range(1)):
                nc.tensor.matmul(
                    ssqp_all[:, g * tpg + t: g * tpg + t + 1],
                    lhsT=x2[:, t * TILE:(t + 1) * TILE],
                    rhs=ones_col_f16[:],
                    start=True, stop=True, skip_group_check=True,
                )

        # ---------------- batched norm ----------------
        normv_all = sm_pool.tile([128, 128], F32, tag="normv")
        nc.scalar.activation(normv_all[:, 0:n_groups * tpg],
                             ssqp_all[:, 0:n_groups * tpg], func=AF.Sqrt)
        rnorm_all = sm_pool.tile([128, 128], F32, tag="rnorm")
        nc.vector.reciprocal(rnorm_all[:, 0:n_groups * tpg],
                             normv_all[:, 0:n_groups * tpg])

        ntiles = n_groups * tpg  # <=128
        nrt = nrtA if n % 2 == 0 else nrtB
        if "bias" not in ab:
            ntp = mb_t[0:ntiles, 128:256]
            nc.tensor.matmul(
                ntp, lhsT=normv_all[:, 0:ntiles], rhs=ident_f32[:],
                start=True, stop=True, is_transpose=True,
                skip_group_check=True,
            )
            nc.vector.tensor_copy(nrt[0:ntiles, :], ntp)

        # ---------------- main loop ----------------
        for g in range(n_groups):
            first_g = g == 0
            keep = lambda what: what not in ab or first_g  # noqa: E731

            xTp = xT_pool.tile([128, tpg * 128], F32)
            lgp = lg_pool.tile([128, tpg * K], F32)
            lg_3d = lgp[:].rearrange("p (t x) -> p t x", t=tpg)
            xT_3d = xTp[:].rearrange("p (t x) -> p t x", t=tpg)
            rnorm_g = rnorm_all[:, g * tpg:(g + 1) * tpg]
            normv_g = normv_all[:, g * tpg:(g + 1) * tpg]

            if keep("bias"):
                nc.tensor.matmul(
                    lgp[:],
                    lhsT=nrt[:],
                    rhs=bias_blk[:, g * tpg * K:(g + 1) * tpg * K],
                    start=True, stop=False, skip_group_check=True,
                )
            for t in (range(tpg) if keep("mm1") else range(1)):
                nc.tensor.matmul(
                    xTp[:, t * 128:(t + 1) * 128],
                    lhsT=xh_all[:, g * group + t * TILE:
                                g * group + (t + 1) * TILE],
                    rhs=rhs_const[:, 0:128],
                    start=True, stop=True, skip_group_check=True,
                )
            mm1b_last = (tpg - 1) if keep("mm1") else 0
            for t in (range(tpg) if keep("mm1") else range(1)):
                nc.tensor.matmul(
                    lgp[:, t * K:(t + 1) * K],
                    lhsT=xh_all[:, g * group + t * TILE:
                                g * group + (t + 1) * TILE],
                    rhs=rhs_const[:, 128:192],
                    start=(not keep("bias") and t == 0), stop=(t == mm1b_last),
                    skip_group_check=True,
                )

            # t1 = lgp * rnorm_b  (fuses softmax scale AND bias application)
            t1 = ew_pool.tile([128, tpg * K], F16, tag="t1")
            t1_3d = t1[:].rearrange("p (t x) -> p t x", t=tpg)
            rnorm_b = rnorm_g.unsqueeze(-1).broadcast_to((128, tpg, K))
            if keep("t1"):
                nc.vector.tensor_mul(t1_3d, lg_3d, rnorm_b)
            else:
                nc.vector.tensor_mul(t1[:, 0:8], lgp[:, 0:8], lgp[:, 0:8])

            E = ew_pool.tile([128, tpg * K], F16, tag="E")
            if keep("exp"):
                nc.scalar.activation(E[:], t1[:], func=AF.Exp)
            else:
                nc.scalar.activation(E[:, 0:8], t1[:, 0:8], func=AF.Exp)
            E_3d = E[:].rearrange("p (t x) -> p t x", t=tpg)

            sume = sm_pool.tile([128, tpg], F32, tag="sume")
            if keep("reduce"):
                eng = nc.vector if o["reduce_engine"] == "dve" else nc.gpsimd
                eng.tensor_reduce(sume[:], E_3d, axis=AX.X, op=ALU.add)
            else:
                nc.vector.tensor_reduce(
                    sume[:, 0:1], E_3d[:, 0:1, :], axis=AX.X, op=ALU.add)
            rsum = sm_pool.tile([128, tpg], F32, tag="rsum")
            nc.vector.reciprocal(rsum[:], sume[:])
            qsc = sm_pool.tile([128, tpg], F32, tag="qsc")
            nc.vector.tensor_mul(qsc[:], rsum[:], rnorm_g)

            wt = ew_pool.tile([128, tpg * K], F16, tag="wt")
            if keep("wt"):
                wt_3d = wt[:].rearrange("p (t x) -> p t x", t=tpg)
                q_b = qsc[:].unsqueeze(-1).broadcast_to((128, tpg, K))
                nc.vector.tensor_mul(wt_3d, E_3d, q_b)
            else:
                nc.vector.tensor_mul(wt[:, 0:8], E[:, 0:8], E[:, 0:8])

            xt = xt_pool.tile([128, tpg * 132], F16)
            xt_3d = xt[:].rearrange("p (t x) -> p t x", t=tpg)
            if keep("xtcopy"):
                if xt_dve > 0:
                    nc.vector.tensor_copy(
                        xt_3d[:, :, 0:xt_dve], xT_3d[:, :, 0:xt_dve])
                if xt_dve < 128:
                    nc.scalar.copy(
                        xt_3d[:, :, xt_dve:128], xT_3d[:, :, xt_dve:128])
                nc.vector.tensor_copy(
                    xt_3d[:, :, 128:129], normv_g.unsqueeze(-1))
            else:
                nc.vector.tensor_copy(xt_3d[:, 0:1, 0:129], xT_3d[:, 0:1, 0:129])

            mm2_tiles = range(tpg)
            if "mm2" in ab:
                mm2_tiles = range(1) if (g == 0 or g == n_groups - 1) else range(0)
            for t in mm2_tiles:
                first = (g == 0 and t == 0)
                last = ("mm2" in ab and g == n_groups - 1 and t == 0) or \
                       (g == n_groups - 1 and t == tpg - 1)
                nc.tensor.matmul(
                    vlad_psum,
                    lhsT=wt[:, t * K:(t + 1) * K],
                    rhs=xt[:, t * 132: t * 132 + 129],
                    start=first, stop=last, skip_group_check=True,
                )

        # -------- epilogue for sample n --------
        acs = ep_pool.tile([K, C], F32, tag="acs")
        nc.vector.tensor_scalar_mul(acs[:], cent_sb[:], asum_col)
        v = ep_pool.tile([K, C], F32, tag="v")
        nc.vector.tensor_sub(v[:], vb[0:K, 0:128], acs[:])
        v2 = ep_pool.tile([K, C], F32, tag="v2")
        nc.vector.tensor_mul(v2[:], v[:], v[:])
        ssqv = sm_pool.tile([K, 1], F32, tag="ssqv")
        nc.vector.tensor_reduce(ssqv[:], v2[:], axis=AX.X, op=ALU.add)
        nv = sm_pool.tile([K, 1], F32, tag="nv")
        nc.scalar.activation(nv[:], ssqv[:], func=AF.Sqrt)
        rnv = sm_pool.tile([K, 1], F32, tag="rnv")
        nc.vector.reciprocal(rnv[:], nv[:])
        rnv8 = sm_pool.tile([K, 1], F32, tag="rnv8")
        nc.vector.tensor_scalar_mul(rnv8[:], rnv[:], 0.125)
        o_t = ep_pool.tile([K, C], F32, tag="o")
        nc.vector.tensor_scalar_mul(o_t[:], v[:], rnv8[:])
        nc.sync.dma_start(out=out_ap[n], in_=o_t[:])


def host_consts(fc_w, fc_b, opts=None):
    o = dict(DEFAULT_OPTS)
    if opts:
        o.update(opts)
    tpg = o["group"] // TILE
    rhs_const = np.zeros((128, 192), dtype=np.float16)
    rhs_const[:, 0:128] = np.eye(128, dtype=np.float16)
    rhs_const[:, 128:192] = fc_w.astype(np.float16).T  # [C, K]
    n_groups = S // o["group"]
    bias_blk = np.zeros((128, n_groups * tpg * K), dtype=np.float16)
    b16 = fc_b.reshape(-1).astype(np.float16)
    for g in range(n_groups):
        for t in range(tpg):
            bias_blk[g * tpg + t,
                     (g * tpg + t) * K:(g * tpg + t + 1) * K] = b16
    ident = np.eye(128, dtype=np.float32)
    return dict(rhs_const=rhs_const, bias_blk=bias_blk, ident=ident)


def make_in_maps(opts=None, seed=0):
    rng = np.random.default_rng(seed)
    fc_w = rng.standard_normal((K, C)).astype(np.float32)
    fc_b = rng.standard_normal((K,)).astype(np.float32)
    consts = host_consts(fc_w, fc_b, opts)
    ins = {
        "x": rng.standard_normal((N_PER_CORE, C, S)).astype(np.float32),
        "centroids": rng.random((K, C)).astype(np.float32),
        **consts,
    }
    return [ins] * N_CORES


def kernel(x, fc_w, fc_b, centroids):
    """Full-input entry point: shards over 8 cores, returns [N, K*C] float32."""
    from concourse.bass_utils import run_bass_kernel_spmd

    x = np.ascontiguousarray(np.asarray(x, dtype=np.float32))
    fc_w = np.ascontiguousarray(np.asarray(fc_w, dtype=np.float32))
    fc_b = np.ascontiguousarray(np.asarray(fc_b, dtype=np.float32))
    centroids = np.ascontiguousarray(np.asarray(centroids, dtype=np.float32))

    consts = host_consts(fc_w, fc_b)
    nc = build_nc(N_PER_CORE, S)
    core_ids = list(range(N_CORES))
    in_maps = []
    for i in core_ids:
        in_maps.append({
            "x": x[i * N_PER_CORE:(i + 1) * N_PER_CORE],
            "centroids": centroids,
            **consts,
        })
    last_exc = None
    for attempt in range(4):
        try:
            res = run_bass_kernel_spmd(nc, in_maps, core_ids)
            break
        except Exception as e:  # noqa: BLE001
            last_exc = e
            if attempt == 3:
                raise
            import time as _time
            _time.sleep(45)
    outs = [res.results[i]["out"].reshape(N_PER_CORE, K * C) for i in range(N_CORES)]
    return np.concatenate(outs, axis=0)
